# revision 46
# baseline (speedup 1.0000x reference)
"""Monotonic chunkwise attention (MoChA-style) Trainium2 kernel.

Full-input contract: kernel(**inputs) takes the unsharded numpy inputs and
returns (context, alpha, beta) matching reference.reference(). Internally
shards the batch across 8 NeuronCores (4 batch elements per core), runs one
SPMD Bass/Tile kernel via bass_utils.run_bass_kernel_spmd, and gathers.

Per-core dataflow (4 batch elements, software-pipelined emission so the
in-order PE stream never blocks on a scan chain):
  energies  e^T[a,s] = W^T @ enc^T; enc^T host-pretransposed, fp8-e4m3;
            weights bf16; f32 PSUM accumulation over 4 d-chunks
  tanh      on ACT with fused per-partition bias (dec@V + b), bf16 out
  reduce    per-s-chunk matmuls (tanh-slice stationary, w_eff moving) land
            mono/chunk energies directly in chunk-major [128,16] layout
  sigmoid   via e=exp(-(x+C)), p=1/(1+e), q=e*p -- keeps the whole scan
            chain on the natural_log_exp ACT table (2 table loads/batch)
  cumsum    triangular matmul (bf16 0/1 weights) + column carry: colsum
            matmul -> DVE tensor_tensor_scan -> ones-row outer-product
            accumulate (carry broadcast in f32 for the log-domain cumsum)
  mov.sums  banded matmuls (in-column band + cross-column band)
  context   deferred packed phase: per-batch M=1 matmuls placed in four
            32-column PE groups (tile_position) run concurrently
"""
import sys

if '/opt/trn_rl_repo' not in sys.path:
    sys.path.insert(0, '/opt/trn_rl_repo')

import numpy as np
import ml_dtypes

BF16NP = ml_dtypes.bfloat16
F8NP = ml_dtypes.float8_e4m3

import concourse.bass as bass  # noqa: E402
import concourse.bacc as bacc  # noqa: E402
import concourse.tile as tile  # noqa: E402
from concourse import mybir  # noqa: E402
from concourse import bass_utils  # noqa: E402

F32 = mybir.dt.float32
F8 = mybir.dt.float8e4
BF16 = mybir.dt.bfloat16
AX = mybir.AxisListType
AF = mybir.ActivationFunctionType
OP = mybir.AluOpType

B, S, D, A = 32, 2048, 512, 128
NCORES = 8
BL = B // NCORES          # 4 batch elements per core
NSC = S // 128            # 16 s-chunks (columns of the chunk-major tiles)
ND = D // 128             # 4 d-chunks
SL = 512                  # s-slice for the energy matmuls
NSL = S // SL             # 4 slices

_BUILD_CACHE: dict = {}


def _scan_consts() -> np.ndarray:
    """[128, 641] f32: tri | BLb | BUb | BLf | BUf | ones_col.

    All are lhsT matrices M[k, i] for out[i, c] = sum_k M[k, i] * x[k, c].
    """
    k = np.arange(128)[:, None]
    i = np.arange(128)[None, :]
    tri = (k <= i).astype(np.float32)                      # inclusive cumsum
    blb = ((k <= i) & (k >= i - 7)).astype(np.float32)     # back-7 in-column
    bub = (k >= 121 + i).astype(np.float32)                # back-7 from prev col
    blf = ((k >= i) & (k <= i + 7)).astype(np.float32)     # fwd-7 in-column
    buf_ = (k <= i - 121).astype(np.float32)               # fwd-7 from next col
    ones_col = np.ones((128, 1), np.float32)
    return np.concatenate([tri, blb, bub, blf, buf_, ones_col], axis=1)


def _build(c_m: float, c_c: float, vg_m: float, vg_c: float):
    key = (c_m, c_c, vg_m, vg_c)
    if key in _BUILD_CACHE:
        return _BUILD_CACHE[key]

    nc = bacc.Bacc("TRN2", target_bir_lowering=False, debug=False,
                   num_devices=NCORES)

    enc_nat = nc.dram_tensor("enc_nat", [BL * S, D], BF16, kind="ExternalInput")
    enc_t = nc.dram_tensor("enc_t", [BL * D, S], F8, kind="ExternalInput")
    wt_d = nc.dram_tensor("wt", [128, ND * 256], F8, kind="ExternalInput")
    vt_d = nc.dram_tensor("vt", [128, ND * 256], BF16, kind="ExternalInput")
    cst_d = nc.dram_tensor("cst", [128, 641], BF16, kind="ExternalInput")
    bmc_d = nc.dram_tensor("bmc", [128, 2], F32, kind="ExternalInput")
    vv_d = nc.dram_tensor("vv", [1, 256], F32, kind="ExternalInput")
    dck_d = nc.dram_tensor("dck", [128, ND * BL], BF16, kind="ExternalInput")
    noi_d = nc.dram_tensor("noi", [128, BL * NSC], F32, kind="ExternalInput")
    pa_d = nc.dram_tensor("pa", [128, BL * NSC], F32, kind="ExternalInput")
    alo_d = nc.dram_tensor("alo", [128, BL * NSC], F32, kind="ExternalOutput")
    beo_d = nc.dram_tensor("beo", [128, BL * NSC], F32, kind="ExternalOutput")
    cto_d = nc.dram_tensor("cto", [BL, D], F32, kind="ExternalOutput")

    with tile.TileContext(nc) as tc:
        with (
            tc.tile_pool(name="singles", bufs=1) as singles,
            tc.tile_pool(name="enc", bufs=3) as encp,
            tc.tile_pool(name="nat", bufs=4) as natp,
            tc.tile_pool(name="tanh", bufs=3) as tanhp,
            tc.tile_pool(name="scan", bufs=3) as scanp,
            tc.tile_pool(name="ps_e", bufs=4, space="PSUM") as ps_e,
            tc.tile_pool(name="ps_r", bufs=2, space="PSUM") as ps_r,
            tc.tile_pool(name="ps_s", bufs=1, space="PSUM") as ps_s,
            tc.tile_pool(name="ps_c", bufs=1, space="PSUM") as ps_c,
        ):
            # ---- load constants / params --------------------------------
            wt_sb = singles.tile([128, ND, 256], F8)
            nc.scalar.dma_start(wt_sb, wt_d[:].rearrange("p (k a) -> p k a", k=ND))
            vt_sb = singles.tile([128, ND * 256], BF16)
            nc.scalar.dma_start(vt_sb, vt_d[:])
            dck_sb = singles.tile([128, ND * BL], BF16)
            nc.scalar.dma_start(dck_sb, dck_d[:])
            bmc_sb = singles.tile([128, 2], F32)
            nc.scalar.dma_start(bmc_sb, bmc_d[:])
            vv_sb = singles.tile([1, 256], F32)
            nc.scalar.dma_start(vv_sb, vv_d[:])
            cst_sb = singles.tile([128, 641], BF16)
            nc.scalar.dma_start(cst_sb, cst_d[:])
            noi_sb = singles.tile([128, BL * NSC], F32)
            nc.scalar.dma_start(noi_sb, noi_d[:])
            pa_sb = singles.tile([128, BL * NSC], F32)
            nc.scalar.dma_start(pa_sb, pa_d[:])

            tri = cst_sb[:, 0:128]
            blb = cst_sb[:, 128:256]
            bub = cst_sb[:, 256:384]
            blf = cst_sb[:, 384:512]
            buf_ = cst_sb[:, 512:640]
            ones_col = cst_sb[:, 640:641]
            ones_row = cst_sb[0:1, 0:128]  # row 0 of tri is all ones

            one_sb = singles.tile([1, 1], F32)
            nc.vector.memset(one_sb, 1.0)
            zrow_sb = singles.tile([1, NSC], F32)
            nc.vector.memset(zrow_sb, 0.0)
            # f32 ones-row for the log-cumsum carry broadcast (the carry is
            # ~|Lcs| <= 40; bf16 rounding there would perturb exp(Lcs) by
            # several percent, so that one matmul stays f32)
            onesrow_f32 = singles.tile([1, 128], F32)
            nc.vector.memset(onesrow_f32, 1.0)
            ncm_sb = singles.tile([128, 1], F32)
            nc.vector.memset(ncm_sb, -c_m)
            cc_sb = singles.tile([128, 1], F32)
            nc.vector.memset(cc_sb, c_c)

            # ---- w_eff = vg * vv / ||vv||  (both branches) --------------
            sq_sb = singles.tile([1, 256], F32)
            nc.scalar.activation(sq_sb, vv_sb, AF.Square)
            ss_sb = singles.tile([1, 2], F32)
            for br in range(2):
                nc.vector.tensor_reduce(
                    out=ss_sb[0:1, br:br + 1],
                    in_=sq_sb[0:1, br * 128:(br + 1) * 128],
                    axis=AX.X, op=OP.add)
            rt_sb = singles.tile([1, 2], F32)
            nc.scalar.activation(rt_sb, ss_sb, AF.Sqrt)
            rs_sb = singles.tile([1, 2], F32)
            nc.vector.reciprocal(rs_sb, rt_sb)
            sc_sb = singles.tile([1, 2], F32)
            nc.vector.tensor_scalar_mul(sc_sb[0:1, 0:1], rs_sb[0:1, 0:1], vg_m)
            nc.vector.tensor_scalar_mul(sc_sb[0:1, 1:2], rs_sb[0:1, 1:2], vg_c)
            weffrow = singles.tile([1, 256], F32)
            for br in range(2):
                nc.vector.tensor_scalar_mul(
                    weffrow[0:1, br * 128:(br + 1) * 128],
                    vv_sb[0:1, br * 128:(br + 1) * 128],
                    sc_sb[0:1, br:br + 1])
            weff_sb = singles.tile([128, 2], BF16)
            bias_sb = singles.tile([128, 2 * BL], F32)

            def emit_wcol_mms():
                wcol_ps = ps_s.tile([128, 2], F32, tag="scan")
                for br in range(2):
                    nc.tensor.matmul(
                        wcol_ps[:, br:br + 1],
                        lhsT=weffrow[0:1, br * 128:(br + 1) * 128],
                        rhs=one_sb[0:1, 0:1], start=True, stop=True)
                nc.vector.tensor_copy(weff_sb, wcol_ps)

            def emit_setup_mms():
                # emitted between batch 0's first energy groups and the rest
                # so these don't gate the kernel start on the PE stream
                # bias[a] = dec @ V + b  (per branch, per batch)
                decv_ps = ps_r.tile([128, 2 * BL], F32, tag="red")
                for br in range(2):
                    for k in range(ND):
                        nc.tensor.matmul(
                            decv_ps[:, br * BL:(br + 1) * BL],
                            lhsT=vt_sb[:, k * 256 + br * 128:
                                       k * 256 + (br + 1) * 128],
                            rhs=dck_sb[:, k * BL:(k + 1) * BL],
                            start=(k == 0), stop=(k == ND - 1))
                for br in range(2):
                    nc.vector.tensor_scalar(
                        out=bias_sb[:, br * BL:(br + 1) * BL],
                        in0=decv_ps[:, br * BL:(br + 1) * BL],
                        scalar1=bmc_sb[:, br:br + 1], scalar2=None, op0=OP.add)

            # ---- helper: chunk-major cumsum -----------------------------
            # x_sb must be bf16. carry_f32 keeps the carry-broadcast matmul
            # in f32 (needed for the log-domain cumsum).
            def cumsum_cm(x_sb, carry_f32):
                cs = ps_s.tile([128, NSC], F32, tag="scan")
                nc.tensor.matmul(cs, lhsT=tri, rhs=x_sb, start=True, stop=False)
                col = ps_c.tile([1, 512], F32, tag="small")
                nc.tensor.matmul(col[0:1, 0:NSC], lhsT=ones_col, rhs=x_sb,
                                 start=True, stop=True)
                incl = scanp.tile([1, NSC], F32, tag=f"incl")
                nc.vector.tensor_tensor_scan(
                    incl, col[0:1, 0:NSC], zrow_sb, 0.0, OP.add, OP.bypass)
                excl = scanp.tile([1, NSC], F32 if carry_f32 else BF16,
                                  tag="excl_f" if carry_f32 else "excl_b")
                nc.vector.tensor_sub(excl, incl, col[0:1, 0:NSC])
                nc.tensor.matmul(cs, lhsT=onesrow_f32 if carry_f32 else ones_row,
                                 rhs=excl, start=False, stop=True)
                return cs

            # ---- main pipeline, software-pipelined emission -------------
            # The PE instruction stream is in-order: emitting batch b's scan
            # and context matmuls AFTER batch b+1's energy/reduce matmuls
            # keeps the PE dense (and HAM-warm) while b's serial DVE/ACT
            # scan chain runs.
            def emit_front(b):
                enct = encp.tile([128, ND, S], F8, tag="tk",
                                 name=f"enct_{b}")
                if b == 0:
                    # slice the first load by s so the first matmul group's
                    # inputs land in one small DMA
                    for sl in range(NSL):
                        nc.sync.dma_start(
                            enct[:, :, sl * SL:(sl + 1) * SL],
                            enc_t[b * D:(b + 1) * D, sl * SL:(sl + 1) * SL]
                            .rearrange("(k p) s -> p k s", p=128))
                else:
                    H = S // 2
                    for h in range(2):
                        nc.sync.dma_start(
                            enct[:, :, h * H:(h + 1) * H],
                            enc_t[b * D:(b + 1) * D, h * H:(h + 1) * H]
                            .rearrange("(k p) s -> p k s", p=128))

                tanh_t = [tanhp.tile([128, S], BF16, tag=f"t{br}",
                                     name=f"tanh{br}_{b}")
                          for br in range(2)]

                def energy_group(sl, br):
                    # fp8 DoubleRow: each matmul contracts a 256-deep pair
                    # of adjacent 128-row subtiles (2 MACs/cell/cycle)
                    pe = ps_e.tile([128, SL], F32, tag="e",
                                   name=f"pe{br}_{b}_{sl}")
                    for kk in range(ND // 2):
                        nc.tensor.matmul(
                            pe,
                            lhsT=wt_sb[:, 2 * kk:2 * kk + 2,
                                       br * 128:(br + 1) * 128],
                            rhs=enct[:, 2 * kk:2 * kk + 2,
                                     sl * SL:(sl + 1) * SL],
                            start=(kk == 0), stop=(kk == ND // 2 - 1),
                            perf_mode=mybir.MatmulPerfMode.DoubleRow)
                    return pe

                def tanh_slice(sl, br, pe):
                    nc.scalar.activation(
                        tanh_t[br][:, sl * SL:(sl + 1) * SL], pe, AF.Tanh,
                        bias=bias_sb[:, br * BL + b:br * BL + b + 1])

                if b == 0:
                    # batch 0: first psum groups emitted bare, then the
                    # setup matmuls (their vt/dck loads overlap these
                    # matmuls), then the dependent tanhs -- keeps both the
                    # PE stream start and the dep order correct
                    pes = [energy_group(0, br) for br in range(2)]
                    emit_setup_mms()
                    for br in range(2):
                        tanh_slice(0, br, pes[br])
                    for sl in range(1, NSL):
                        for br in range(2):
                            tanh_slice(sl, br, energy_group(sl, br))
                else:
                    for sl in range(NSL):
                        for br in range(2):
                            tanh_slice(sl, br, energy_group(sl, br))

                if b == 0:
                    emit_wcol_mms()

                # reduce with w_eff -> chunk-major [128, NSC] per branch
                red = [ps_r.tile([128, NSC], F32, tag="red",
                                 name=f"red{br}_{b}")
                       for br in range(2)]
                for br in range(2):
                    for c in range(NSC):
                        nc.tensor.matmul(
                            red[br][:, c:c + 1],
                            lhsT=tanh_t[br][:, c * 128:(c + 1) * 128],
                            rhs=weff_sb[:, br:br + 1], start=True, stop=True)

                # p_in / eu read the red psums as early as possible so the
                # red slots recycle for the next batch
                p_in = scanp.tile([128, NSC], F32, tag="p_in",
                                  name=f"p_in_{b}")
                nc.vector.tensor_add(p_in, red[0],
                                     noi_sb[:, b * NSC:(b + 1) * NSC])
                eu = scanp.tile([128, NSC], BF16, tag="eu", name=f"eu_{b}")
                nc.scalar.activation(eu, red[1], AF.Exp, bias=cc_sb[:, 0:1])

                # scan-chain ACT/DVE prefix emitted in front(b) so it runs
                # BEFORE the next batch's tanh block on the in-order ACT
                # stream. Sigmoid lives in a different ACT table than Exp/Ln;
                #   e = exp(-(x+C)), p_select = 1/(1+e), q = 1-p = e*p_select
                # keeps the chain on natural_log_exp_and_others
                # (Tanh -> Exp/Ln = 2 table switches per batch).
                e_ng = scanp.tile([128, NSC], F32, tag="e_ng")
                nc.scalar.activation(e_ng, p_in, AF.Exp, scale=-1.0,
                                     bias=ncm_sb[:, 0:1])
                e1 = scanp.tile([128, NSC], F32, tag="e1")
                nc.vector.tensor_scalar_add(e1, e_ng, 1.0)
                p_sel = scanp.tile([128, NSC], F32, tag="p_sel")
                nc.vector.reciprocal(p_sel, e1)
                q = scanp.tile([128, NSC], F32, tag="q")
                nc.vector.tensor_mul(q, e_ng, p_sel)
                lq = scanp.tile([128, NSC], BF16, tag="lq")
                nc.scalar.activation(lq, q, AF.Ln)
                return p_sel, eu, lq

            def emit_cs1(b, p_sel, lq):
                # cumsum #1 + its exps, emitted right after front(b): the
                # cp/rcp ACT ops land BEFORE batch b+1's tanh block on the
                # in-order ACT stream, keeping the scan chain short.
                lcs = cumsum_cm(lq, True)
                cp = scanp.tile([128, NSC], F32, tag="cp")
                nc.scalar.activation(cp, lcs, AF.Exp)
                rcp = scanp.tile([128, NSC], F32, tag="rcp")
                nc.scalar.activation(rcp, lcs, AF.Exp, scale=-1.0)
                t_sb = scanp.tile([128, NSC], BF16, tag="t_sb")
                nc.vector.tensor_mul(t_sb, pa_sb[:, b * NSC:(b + 1) * NSC], rcp)
                return cp, t_sb

            def load_nat(b):
                # natural-layout enc for the final context phase; emitted
                # after batch b+1's encT loads in sync-ring order
                nat = natp.tile([128, NSC, SL], BF16, tag="nat",
                                name=f"nat_{b}")
                nc.sync.dma_start(
                    nat, enc_nat[b * S:(b + 1) * S, :].rearrange(
                        "(c p) d -> p c d", p=128))
                return nat

            def emit_back(b, p_sel, eu, cp, t_sb, nat):
                # denom first (inputs long ready) -- its matmuls cover the
                # reciprocal/mult latency before ms2 below
                den = ps_s.tile([128, NSC], F32, tag="scan")
                nc.tensor.matmul(den, lhsT=blb, rhs=eu, start=True, stop=False)
                nc.tensor.matmul(den[:, 1:NSC], lhsT=bub, rhs=eu[:, 0:NSC - 1],
                                 start=False, stop=True)
                dinv = scanp.tile([128, NSC], F32, tag="dinv")
                nc.vector.reciprocal(dinv, den)

                ct = cumsum_cm(t_sb, False)

                a1 = scanp.tile([128, NSC], F32, tag="a1")
                nc.vector.tensor_mul(a1, p_sel, cp)
                alpha = scanp.tile([128, NSC], F32, tag="alpha")
                nc.vector.tensor_mul(alpha, a1, ct)
                nc.scalar.dma_start(alo_d[:, b * NSC:(b + 1) * NSC], alpha)

                r1 = scanp.tile([128, NSC], BF16, tag="r1")
                nc.vector.tensor_mul(r1, alpha, dinv)

                # ms2 = moving_sum(r1, forward=7)
                ms2 = ps_s.tile([128, NSC], F32, tag="scan")
                nc.tensor.matmul(ms2, lhsT=blf, rhs=r1, start=True, stop=False)
                nc.tensor.matmul(ms2[:, 0:NSC - 1], lhsT=buf_, rhs=r1[:, 1:NSC],
                                 start=False, stop=True)
                beta = scanp.tile([128, NSC], F32, tag="beta")
                nc.vector.tensor_mul(beta, eu, ms2)
                nc.scalar.dma_start(beo_d[:, b * NSC:(b + 1) * NSC], beta)
                beta_bf = scanp.tile([128, NSC], BF16, tag=f"beta_bf{b}",
                                     name=f"betabf_{b}")
                nc.vector.tensor_copy(beta_bf, beta)
                return beta_bf, nat

            # Context matmuls are M=1 (1/128 of the PE array); packing
            # batches into separate 32-column groups at psum partition 32*b
            # runs them concurrently. Batches 0-2 are emitted as a 3-way
            # packed phase overlapping batch 3's front; batch 3 finishes
            # after its beta.
            ctx_ps = None

            def emit_ctx(parts):
                ctx_ps = ps_c.tile([128, 512], F32, tag="small")
                for c in range(NSC):
                    for b, beta_bf, nat in parts:
                        nc.tensor.matmul(
                            ctx_ps[32 * b:32 * b + 1, :],
                            lhsT=beta_bf[:, c:c + 1], rhs=nat[:, c, :],
                            start=(c == 0), stop=(c == NSC - 1),
                            skip_group_check=True,
                            tile_position=(0, 32 * b))
                return ctx_ps

            pending = None
            ctx_parts = []
            nat3 = None
            for b in range(BL):
                p_sel, eu, lq = emit_front(b)
                if b == BL - 1:
                    # last batch: its nat load goes on the ring right after
                    # its encT so it is resident well before the ctx phase
                    nat3 = load_nat(b)
                cp, t_sb = emit_cs1(b, p_sel, lq)
                if pending is not None:
                    bb = pending[0]
                    nat_b = load_nat(bb)
                    beta_bf, nat = emit_back(*pending, nat_b)
                    ctx_parts.append((bb, beta_bf, nat))
                pending = (b, p_sel, eu, cp, t_sb)
            bb = pending[0]
            beta_bf, nat = emit_back(*pending, nat3)
            ctx_parts.append((bb, beta_bf, nat))

            # ---- packed context phase: 4 batches in 4 PE column-groups ---
            ctx_ps = emit_ctx(ctx_parts)

            ctx_sb = scanp.tile([128, 512], F32, tag="ctx_sb")
            nc.vector.tensor_copy(ctx_sb, ctx_ps)
            for b in range(BL):
                nc.scalar.dma_start(cto_d[b:b + 1, :],
                                    ctx_sb[32 * b:32 * b + 1, :])

    nc.compile()
    _BUILD_CACHE[key] = nc
    return nc


def _prep_core_inputs(enc, dec, pa, noise, host_consts):
    """Per-core (BL batches) input map. enc [BL,S,D] f32, dec [BL,512]."""
    encb = np.ascontiguousarray(enc.astype(BF16NP))
    enct = np.ascontiguousarray(enc.transpose(0, 2, 1).astype(F8NP))
    # dck[p, k*BL+b] = dec[b, k*128+p]
    dck = np.ascontiguousarray(
        dec.reshape(BL, ND, 128).transpose(2, 1, 0).reshape(128, ND * BL))
    # chunk-major [128, BL*NSC], col = b*NSC + c, value at s = c*128+p
    def cm(x):  # x [BL, S]
        return np.ascontiguousarray(
            x.reshape(BL, NSC, 128).transpose(2, 0, 1).reshape(128, BL * NSC))
    m = {
        "enc_nat": encb.reshape(BL * S, D),
        "enc_t": enct.reshape(BL * D, S),
        "noi": cm(noise).astype(np.float32),
        "pa": cm(pa).astype(np.float32),
        "dck": dck.astype(BF16NP),
    }
    m.update(host_consts)
    return m


def kernel(encoder_outputs, decoder_h, prev_alpha, noise,
           mW, mV, mb, mvv, mvg, mvb, mr,
           cW, cV, cb, cvv, cvg, cvb, cr):
    encoder_outputs = np.asarray(encoder_outputs, np.float32)
    dec = np.asarray(decoder_h, np.float32)[:, 0, :]
    prev_alpha = np.asarray(prev_alpha, np.float32)
    noise = np.asarray(noise, np.float32)

    c_m = float(np.asarray(mvb)[0] + np.asarray(mr)[0])
    c_c = float(np.asarray(cvb)[0] + np.asarray(cr)[0])
    vg_m = float(np.asarray(mvg)[0])
    vg_c = float(np.asarray(cvg)[0])

    nc = _build(c_m, c_c, vg_m, vg_c)

    # weights: wT[p, k*256 + br*128 + a] = W_br[a, k*128+p]
    def packT(wm, wc, dtype):
        out = np.empty((128, ND, 256), np.float32)
        wmT = np.asarray(wm, np.float32).T  # [D, A]
        wcT = np.asarray(wc, np.float32).T
        for k in range(ND):
            out[:, k, 0:128] = wmT[k * 128:(k + 1) * 128]
            out[:, k, 128:256] = wcT[k * 128:(k + 1) * 128]
        return np.ascontiguousarray(out.reshape(128, ND * 256).astype(dtype))

    host_consts = {
        "wt": packT(mW, cW, F8NP),
        "vt": packT(mV, cV, BF16NP),
        "cst": _scan_consts().astype(BF16NP),
        "bmc": np.ascontiguousarray(
            np.stack([np.asarray(mb, np.float32),
                      np.asarray(cb, np.float32)], axis=1)),
        "vv": np.ascontiguousarray(
            np.concatenate([np.asarray(mvv, np.float32)[0],
                            np.asarray(cvv, np.float32)[0]])[None, :]),
    }

    in_maps = []
    for i in range(NCORES):
        sl = slice(i * BL, (i + 1) * BL)
        in_maps.append(_prep_core_inputs(
            encoder_outputs[sl], dec[sl], prev_alpha[sl], noise[sl],
            host_consts))

    global LAST_RESULT
    res = bass_utils.run_bass_kernel_spmd(
        nc, in_maps, core_ids=list(range(NCORES)), trace=TRACE,
        **RUN_KWARGS)
    LAST_RESULT = res

    ctx = np.empty((B, D), np.float32)
    alpha = np.empty((B, S), np.float32)
    beta = np.empty((B, S), np.float32)
    for i in range(NCORES):
        r = res.results[i]
        ctx[i * BL:(i + 1) * BL] = r["cto"]
        # [128, BL*NSC] -> [BL, S]: value at (p, b*NSC+c) is s = c*128+p
        for name, dst in (("alo", alpha), ("beo", beta)):
            x = r[name].reshape(128, BL, NSC).transpose(1, 2, 0).reshape(BL, S)
            dst[i * BL:(i + 1) * BL] = x
    return ctx, alpha, beta


TRACE = False
RUN_KWARGS: dict = {}
LAST_RESULT = None


# revision 47
# speedup vs baseline: 1.0589x; 1.0589x over previous
"""Monotonic chunkwise attention (MoChA-style) Trainium2 kernel.

Full-input contract: kernel(**inputs) takes the unsharded numpy inputs and
returns (context, alpha, beta) matching reference.reference(). Internally
shards the batch across 8 NeuronCores (4 batch elements per core), runs one
SPMD Bass/Tile kernel via bass_utils.run_bass_kernel_spmd, and gathers.

Per-core dataflow (4 batch elements, software-pipelined emission so the
in-order PE stream never blocks on a scan chain):
  energies  e^T[a,s] = W^T @ enc^T; enc^T host-pretransposed, fp8-e4m3;
            weights bf16; f32 PSUM accumulation over 4 d-chunks
  tanh      on ACT with fused per-partition bias (dec@V + b), bf16 out
  reduce    per-s-chunk matmuls (tanh-slice stationary, w_eff moving) land
            mono/chunk energies directly in chunk-major [128,16] layout
  sigmoid   via e=exp(-(x+C)), p=1/(1+e), q=e*p -- keeps the whole scan
            chain on the natural_log_exp ACT table (2 table loads/batch)
  cumsum    triangular matmul (bf16 0/1 weights) + column carry: colsum
            matmul -> DVE tensor_tensor_scan -> ones-row outer-product
            accumulate (carry broadcast in f32 for the log-domain cumsum)
  mov.sums  banded matmuls (in-column band + cross-column band)
  context   deferred packed phase: per-batch M=1 matmuls placed in four
            32-column PE groups (tile_position) run concurrently
"""
import sys

if '/opt/trn_rl_repo' not in sys.path:
    sys.path.insert(0, '/opt/trn_rl_repo')

import numpy as np
import ml_dtypes

BF16NP = ml_dtypes.bfloat16
F8NP = ml_dtypes.float8_e4m3

import concourse.bass as bass  # noqa: E402
import concourse.bacc as bacc  # noqa: E402
import concourse.tile as tile  # noqa: E402
from concourse import mybir  # noqa: E402
from concourse import bass_utils  # noqa: E402

F32 = mybir.dt.float32
F8 = mybir.dt.float8e4
BF16 = mybir.dt.bfloat16
AX = mybir.AxisListType
AF = mybir.ActivationFunctionType
OP = mybir.AluOpType

B, S, D, A = 32, 2048, 512, 128
NCORES = 8
BL = B // NCORES          # 4 batch elements per core
NSC = S // 128            # 16 s-chunks (columns of the chunk-major tiles)
ND = D // 128             # 4 d-chunks
SL = 512                  # s-slice for the energy matmuls
NSL = S // SL             # 4 slices

_BUILD_CACHE: dict = {}


def _scan_consts() -> np.ndarray:
    """[128, 641] f32: tri | BLb | BUb | BLf | BUf | ones_col.

    All are lhsT matrices M[k, i] for out[i, c] = sum_k M[k, i] * x[k, c].
    """
    k = np.arange(128)[:, None]
    i = np.arange(128)[None, :]
    tri = (k <= i).astype(np.float32)                      # inclusive cumsum
    blb = ((k <= i) & (k >= i - 7)).astype(np.float32)     # back-7 in-column
    bub = (k >= 121 + i).astype(np.float32)                # back-7 from prev col
    blf = ((k >= i) & (k <= i + 7)).astype(np.float32)     # fwd-7 in-column
    buf_ = (k <= i - 121).astype(np.float32)               # fwd-7 from next col
    ones_col = np.ones((128, 1), np.float32)
    return np.concatenate([tri, blb, bub, blf, buf_, ones_col], axis=1)


def _build(c_m: float, c_c: float, vg_m: float, vg_c: float):
    key = (c_m, c_c, vg_m, vg_c)
    if key in _BUILD_CACHE:
        return _BUILD_CACHE[key]

    nc = bacc.Bacc("TRN2", target_bir_lowering=False, debug=False,
                   num_devices=NCORES)

    enc_nat = nc.dram_tensor("enc_nat", [BL * S, D], BF16, kind="ExternalInput")
    enc_t = nc.dram_tensor("enc_t", [BL * D, S], F8, kind="ExternalInput")
    wt_d = nc.dram_tensor("wt", [128, ND * 256], BF16, kind="ExternalInput")
    vt_d = nc.dram_tensor("vt", [128, ND * 256], BF16, kind="ExternalInput")
    cst_d = nc.dram_tensor("cst", [128, 641], BF16, kind="ExternalInput")
    bmc_d = nc.dram_tensor("bmc", [128, 2], F32, kind="ExternalInput")
    vv_d = nc.dram_tensor("vv", [1, 256], F32, kind="ExternalInput")
    dck_d = nc.dram_tensor("dck", [128, ND * BL], BF16, kind="ExternalInput")
    noi_d = nc.dram_tensor("noi", [128, BL * NSC], F32, kind="ExternalInput")
    pa_d = nc.dram_tensor("pa", [128, BL * NSC], F32, kind="ExternalInput")
    alo_d = nc.dram_tensor("alo", [128, BL * NSC], F32, kind="ExternalOutput")
    beo_d = nc.dram_tensor("beo", [128, BL * NSC], F32, kind="ExternalOutput")
    cto_d = nc.dram_tensor("cto", [BL, D], F32, kind="ExternalOutput")

    with tile.TileContext(nc) as tc:
        with (
            tc.tile_pool(name="singles", bufs=1) as singles,
            tc.tile_pool(name="enc", bufs=3) as encp,
            tc.tile_pool(name="nat", bufs=4) as natp,
            tc.tile_pool(name="tanh", bufs=3) as tanhp,
            tc.tile_pool(name="scan", bufs=3) as scanp,
            tc.tile_pool(name="ps_e", bufs=4, space="PSUM") as ps_e,
            tc.tile_pool(name="ps_r", bufs=2, space="PSUM") as ps_r,
            tc.tile_pool(name="ps_s", bufs=1, space="PSUM") as ps_s,
            tc.tile_pool(name="ps_c", bufs=1, space="PSUM") as ps_c,
        ):
            # ---- load constants / params --------------------------------
            wt_sb = singles.tile([128, ND, 256], BF16)
            nc.scalar.dma_start(wt_sb, wt_d[:].rearrange("p (k a) -> p k a", k=ND))
            vt_sb = singles.tile([128, ND * 256], BF16)
            nc.scalar.dma_start(vt_sb, vt_d[:])
            dck_sb = singles.tile([128, ND * BL], BF16)
            nc.scalar.dma_start(dck_sb, dck_d[:])
            bmc_sb = singles.tile([128, 2], F32)
            nc.scalar.dma_start(bmc_sb, bmc_d[:])
            vv_sb = singles.tile([1, 256], F32)
            nc.scalar.dma_start(vv_sb, vv_d[:])
            cst_sb = singles.tile([128, 641], BF16)
            nc.scalar.dma_start(cst_sb, cst_d[:])
            noi_sb = singles.tile([128, BL * NSC], F32)
            nc.scalar.dma_start(noi_sb, noi_d[:])
            pa_sb = singles.tile([128, BL * NSC], F32)
            nc.scalar.dma_start(pa_sb, pa_d[:])

            tri = cst_sb[:, 0:128]
            blb = cst_sb[:, 128:256]
            bub = cst_sb[:, 256:384]
            blf = cst_sb[:, 384:512]
            buf_ = cst_sb[:, 512:640]
            ones_col = cst_sb[:, 640:641]
            ones_row = cst_sb[0:1, 0:128]  # row 0 of tri is all ones

            one_sb = singles.tile([1, 1], F32)
            nc.vector.memset(one_sb, 1.0)
            zrow_sb = singles.tile([1, NSC], F32)
            nc.vector.memset(zrow_sb, 0.0)
            # f32 ones-row for the log-cumsum carry broadcast (the carry is
            # ~|Lcs| <= 40; bf16 rounding there would perturb exp(Lcs) by
            # several percent, so that one matmul stays f32)
            onesrow_f32 = singles.tile([1, 128], F32)
            nc.vector.memset(onesrow_f32, 1.0)
            ncm_sb = singles.tile([128, 1], F32)
            nc.vector.memset(ncm_sb, -c_m)
            cc_sb = singles.tile([128, 1], F32)
            nc.vector.memset(cc_sb, c_c)

            # ---- w_eff = vg * vv / ||vv||  (both branches) --------------
            sq_sb = singles.tile([1, 256], F32)
            nc.scalar.activation(sq_sb, vv_sb, AF.Square)
            ss_sb = singles.tile([1, 2], F32)
            for br in range(2):
                nc.vector.tensor_reduce(
                    out=ss_sb[0:1, br:br + 1],
                    in_=sq_sb[0:1, br * 128:(br + 1) * 128],
                    axis=AX.X, op=OP.add)
            rt_sb = singles.tile([1, 2], F32)
            nc.scalar.activation(rt_sb, ss_sb, AF.Sqrt)
            rs_sb = singles.tile([1, 2], F32)
            nc.vector.reciprocal(rs_sb, rt_sb)
            sc_sb = singles.tile([1, 2], F32)
            nc.vector.tensor_scalar_mul(sc_sb[0:1, 0:1], rs_sb[0:1, 0:1], vg_m)
            nc.vector.tensor_scalar_mul(sc_sb[0:1, 1:2], rs_sb[0:1, 1:2], vg_c)
            weffrow = singles.tile([1, 256], F32)
            for br in range(2):
                nc.vector.tensor_scalar_mul(
                    weffrow[0:1, br * 128:(br + 1) * 128],
                    vv_sb[0:1, br * 128:(br + 1) * 128],
                    sc_sb[0:1, br:br + 1])
            weff_sb = singles.tile([128, 2], BF16)
            bias_sb = singles.tile([128, 2 * BL], F32)

            def emit_wcol_mms():
                wcol_ps = ps_s.tile([128, 2], F32, tag="scan")
                for br in range(2):
                    nc.tensor.matmul(
                        wcol_ps[:, br:br + 1],
                        lhsT=weffrow[0:1, br * 128:(br + 1) * 128],
                        rhs=one_sb[0:1, 0:1], start=True, stop=True)
                nc.vector.tensor_copy(weff_sb, wcol_ps)

            def emit_setup_mms():
                # emitted between batch 0's first energy groups and the rest
                # so these don't gate the kernel start on the PE stream
                # bias[a] = dec @ V + b  (per branch, per batch)
                decv_ps = ps_r.tile([128, 2 * BL], F32, tag="red")
                for br in range(2):
                    for k in range(ND):
                        nc.tensor.matmul(
                            decv_ps[:, br * BL:(br + 1) * BL],
                            lhsT=vt_sb[:, k * 256 + br * 128:
                                       k * 256 + (br + 1) * 128],
                            rhs=dck_sb[:, k * BL:(k + 1) * BL],
                            start=(k == 0), stop=(k == ND - 1))
                for br in range(2):
                    nc.vector.tensor_scalar(
                        out=bias_sb[:, br * BL:(br + 1) * BL],
                        in0=decv_ps[:, br * BL:(br + 1) * BL],
                        scalar1=bmc_sb[:, br:br + 1], scalar2=None, op0=OP.add)

            # ---- helper: chunk-major cumsum -----------------------------
            # x_sb must be bf16. carry_f32 keeps the carry-broadcast matmul
            # in f32 (needed for the log-domain cumsum).
            def cumsum_cm(x_sb, carry_f32):
                cs = ps_s.tile([128, NSC], F32, tag="scan")
                nc.tensor.matmul(cs, lhsT=tri, rhs=x_sb, start=True, stop=False)
                col = ps_c.tile([1, 512], F32, tag="small")
                nc.tensor.matmul(col[0:1, 0:NSC], lhsT=ones_col, rhs=x_sb,
                                 start=True, stop=True)
                incl = scanp.tile([1, NSC], F32, tag=f"incl")
                nc.vector.tensor_tensor_scan(
                    incl, col[0:1, 0:NSC], zrow_sb, 0.0, OP.add, OP.bypass)
                excl = scanp.tile([1, NSC], F32 if carry_f32 else BF16,
                                  tag="excl_f" if carry_f32 else "excl_b")
                nc.vector.tensor_sub(excl, incl, col[0:1, 0:NSC])
                nc.tensor.matmul(cs, lhsT=onesrow_f32 if carry_f32 else ones_row,
                                 rhs=excl, start=False, stop=True)
                return cs

            # ---- main pipeline, software-pipelined emission -------------
            # The PE instruction stream is in-order: emitting batch b's scan
            # and context matmuls AFTER batch b+1's energy/reduce matmuls
            # keeps the PE dense (and HAM-warm) while b's serial DVE/ACT
            # scan chain runs.
            def emit_front(b):
                enct = encp.tile([128, ND, S], F8, tag="tk",
                                 name=f"enct_{b}")
                if b == 0:
                    # slice the first load by s so the first matmul group's
                    # inputs land in one small DMA
                    for sl in range(NSL):
                        nc.sync.dma_start(
                            enct[:, :, sl * SL:(sl + 1) * SL],
                            enc_t[b * D:(b + 1) * D, sl * SL:(sl + 1) * SL]
                            .rearrange("(k p) s -> p k s", p=128))
                else:
                    H = S // 2
                    for h in range(2):
                        nc.sync.dma_start(
                            enct[:, :, h * H:(h + 1) * H],
                            enc_t[b * D:(b + 1) * D, h * H:(h + 1) * H]
                            .rearrange("(k p) s -> p k s", p=128))

                tanh_t = [tanhp.tile([128, S], BF16, tag=f"t{br}",
                                     name=f"tanh{br}_{b}")
                          for br in range(2)]

                def energy_group(sl, br):
                    pe = ps_e.tile([128, SL], F32, tag="e",
                                   name=f"pe{br}_{b}_{sl}")
                    for k in range(ND):
                        nc.tensor.matmul(
                            pe,
                            lhsT=wt_sb[:, k, br * 128:(br + 1) * 128],
                            rhs=enct[:, k, sl * SL:(sl + 1) * SL],
                            start=(k == 0), stop=(k == ND - 1))
                    return pe

                def tanh_slice(sl, br, pe):
                    nc.scalar.activation(
                        tanh_t[br][:, sl * SL:(sl + 1) * SL], pe, AF.Tanh,
                        bias=bias_sb[:, br * BL + b:br * BL + b + 1])

                if b == 0:
                    # batch 0: first psum groups emitted bare, then the
                    # setup matmuls (their vt/dck loads overlap these
                    # matmuls), then the dependent tanhs -- keeps both the
                    # PE stream start and the dep order correct
                    pes = [energy_group(0, br) for br in range(2)]
                    emit_setup_mms()
                    for br in range(2):
                        tanh_slice(0, br, pes[br])
                    for sl in range(1, NSL):
                        for br in range(2):
                            tanh_slice(sl, br, energy_group(sl, br))
                else:
                    for sl in range(NSL):
                        for br in range(2):
                            tanh_slice(sl, br, energy_group(sl, br))

                if b == 0:
                    emit_wcol_mms()

                # reduce with w_eff -> chunk-major [128, NSC] per branch
                red = [ps_r.tile([128, NSC], F32, tag="red",
                                 name=f"red{br}_{b}")
                       for br in range(2)]
                for br in range(2):
                    for c in range(NSC):
                        nc.tensor.matmul(
                            red[br][:, c:c + 1],
                            lhsT=tanh_t[br][:, c * 128:(c + 1) * 128],
                            rhs=weff_sb[:, br:br + 1], start=True, stop=True)

                # p_in / eu read the red psums as early as possible so the
                # red slots recycle for the next batch
                p_in = scanp.tile([128, NSC], F32, tag="p_in",
                                  name=f"p_in_{b}")
                nc.vector.tensor_add(p_in, red[0],
                                     noi_sb[:, b * NSC:(b + 1) * NSC])
                eu = scanp.tile([128, NSC], BF16, tag="eu", name=f"eu_{b}")
                nc.scalar.activation(eu, red[1], AF.Exp, bias=cc_sb[:, 0:1])

                # scan-chain ACT/DVE prefix emitted in front(b) so it runs
                # BEFORE the next batch's tanh block on the in-order ACT
                # stream. Sigmoid lives in a different ACT table than Exp/Ln;
                #   e = exp(-(x+C)), p_select = 1/(1+e), q = 1-p = e*p_select
                # keeps the chain on natural_log_exp_and_others
                # (Tanh -> Exp/Ln = 2 table switches per batch).
                e_ng = scanp.tile([128, NSC], F32, tag="e_ng")
                nc.scalar.activation(e_ng, p_in, AF.Exp, scale=-1.0,
                                     bias=ncm_sb[:, 0:1])
                e1 = scanp.tile([128, NSC], F32, tag="e1")
                nc.vector.tensor_scalar_add(e1, e_ng, 1.0)
                p_sel = scanp.tile([128, NSC], F32, tag="p_sel")
                nc.vector.reciprocal(p_sel, e1)
                q = scanp.tile([128, NSC], F32, tag="q")
                nc.vector.tensor_mul(q, e_ng, p_sel)
                lq = scanp.tile([128, NSC], BF16, tag="lq")
                nc.scalar.activation(lq, q, AF.Ln)
                return p_sel, eu, lq

            def emit_cs1(b, p_sel, lq):
                # cumsum #1 + its exps, emitted right after front(b): the
                # cp/rcp ACT ops land BEFORE batch b+1's tanh block on the
                # in-order ACT stream, keeping the scan chain short.
                lcs = cumsum_cm(lq, True)
                cp = scanp.tile([128, NSC], F32, tag="cp")
                nc.scalar.activation(cp, lcs, AF.Exp)
                rcp = scanp.tile([128, NSC], F32, tag="rcp")
                nc.scalar.activation(rcp, lcs, AF.Exp, scale=-1.0)
                t_sb = scanp.tile([128, NSC], BF16, tag="t_sb")
                nc.vector.tensor_mul(t_sb, pa_sb[:, b * NSC:(b + 1) * NSC], rcp)
                return cp, t_sb

            def load_nat(b):
                # natural-layout enc for the final context phase; emitted
                # after batch b+1's encT loads in sync-ring order
                nat = natp.tile([128, NSC, SL], BF16, tag="nat",
                                name=f"nat_{b}")
                nc.sync.dma_start(
                    nat, enc_nat[b * S:(b + 1) * S, :].rearrange(
                        "(c p) d -> p c d", p=128))
                return nat

            def emit_back(b, p_sel, eu, cp, t_sb, nat):
                # denom first (inputs long ready) -- its matmuls cover the
                # reciprocal/mult latency before ms2 below
                den = ps_s.tile([128, NSC], F32, tag="scan")
                nc.tensor.matmul(den, lhsT=blb, rhs=eu, start=True, stop=False)
                nc.tensor.matmul(den[:, 1:NSC], lhsT=bub, rhs=eu[:, 0:NSC - 1],
                                 start=False, stop=True)
                dinv = scanp.tile([128, NSC], F32, tag="dinv")
                nc.vector.reciprocal(dinv, den)

                ct = cumsum_cm(t_sb, False)

                a1 = scanp.tile([128, NSC], F32, tag="a1")
                nc.vector.tensor_mul(a1, p_sel, cp)
                alpha = scanp.tile([128, NSC], F32, tag="alpha")
                nc.vector.tensor_mul(alpha, a1, ct)
                nc.scalar.dma_start(alo_d[:, b * NSC:(b + 1) * NSC], alpha)

                r1 = scanp.tile([128, NSC], BF16, tag="r1")
                nc.vector.tensor_mul(r1, alpha, dinv)

                # ms2 = moving_sum(r1, forward=7)
                ms2 = ps_s.tile([128, NSC], F32, tag="scan")
                nc.tensor.matmul(ms2, lhsT=blf, rhs=r1, start=True, stop=False)
                nc.tensor.matmul(ms2[:, 0:NSC - 1], lhsT=buf_, rhs=r1[:, 1:NSC],
                                 start=False, stop=True)
                beta = scanp.tile([128, NSC], F32, tag="beta")
                nc.vector.tensor_mul(beta, eu, ms2)
                nc.scalar.dma_start(beo_d[:, b * NSC:(b + 1) * NSC], beta)
                beta_bf = scanp.tile([128, NSC], BF16, tag=f"beta_bf{b}",
                                     name=f"betabf_{b}")
                nc.vector.tensor_copy(beta_bf, beta)
                return beta_bf, nat

            # Context matmuls are M=1 (1/128 of the PE array); packing
            # batches into separate 32-column groups at psum partition 32*b
            # runs them concurrently. Batches 0-2 are emitted as a 3-way
            # packed phase overlapping batch 3's front; batch 3 finishes
            # after its beta.
            ctx_ps = None

            def emit_ctx(parts):
                ctx_ps = ps_c.tile([128, 512], F32, tag="small")
                for c in range(NSC):
                    for b, beta_bf, nat in parts:
                        nc.tensor.matmul(
                            ctx_ps[32 * b:32 * b + 1, :],
                            lhsT=beta_bf[:, c:c + 1], rhs=nat[:, c, :],
                            start=(c == 0), stop=(c == NSC - 1),
                            skip_group_check=True,
                            tile_position=(0, 32 * b))
                return ctx_ps

            pending = None
            ctx_parts = []
            nat3 = None
            for b in range(BL):
                p_sel, eu, lq = emit_front(b)
                if b == BL - 1:
                    # last batch: its nat load goes on the ring right after
                    # its encT so it is resident well before the ctx phase
                    nat3 = load_nat(b)
                cp, t_sb = emit_cs1(b, p_sel, lq)
                if pending is not None:
                    bb = pending[0]
                    nat_b = load_nat(bb)
                    beta_bf, nat = emit_back(*pending, nat_b)
                    ctx_parts.append((bb, beta_bf, nat))
                pending = (b, p_sel, eu, cp, t_sb)
            bb = pending[0]
            beta_bf, nat = emit_back(*pending, nat3)
            ctx_parts.append((bb, beta_bf, nat))

            # ---- packed context phase: 4 batches in 4 PE column-groups ---
            ctx_ps = emit_ctx(ctx_parts)

            ctx_sb = scanp.tile([128, 512], F32, tag="ctx_sb")
            nc.vector.tensor_copy(ctx_sb, ctx_ps)
            for b in range(BL):
                nc.scalar.dma_start(cto_d[b:b + 1, :],
                                    ctx_sb[32 * b:32 * b + 1, :])

    nc.compile()
    _BUILD_CACHE[key] = nc
    return nc


def _prep_core_inputs(enc, dec, pa, noise, host_consts):
    """Per-core (BL batches) input map. enc [BL,S,D] f32, dec [BL,512]."""
    encb = np.ascontiguousarray(enc.astype(BF16NP))
    enct = np.ascontiguousarray(enc.transpose(0, 2, 1).astype(F8NP))
    # dck[p, k*BL+b] = dec[b, k*128+p]
    dck = np.ascontiguousarray(
        dec.reshape(BL, ND, 128).transpose(2, 1, 0).reshape(128, ND * BL))
    # chunk-major [128, BL*NSC], col = b*NSC + c, value at s = c*128+p
    def cm(x):  # x [BL, S]
        return np.ascontiguousarray(
            x.reshape(BL, NSC, 128).transpose(2, 0, 1).reshape(128, BL * NSC))
    m = {
        "enc_nat": encb.reshape(BL * S, D),
        "enc_t": enct.reshape(BL * D, S),
        "noi": cm(noise).astype(np.float32),
        "pa": cm(pa).astype(np.float32),
        "dck": dck.astype(BF16NP),
    }
    m.update(host_consts)
    return m


def kernel(encoder_outputs, decoder_h, prev_alpha, noise,
           mW, mV, mb, mvv, mvg, mvb, mr,
           cW, cV, cb, cvv, cvg, cvb, cr):
    encoder_outputs = np.asarray(encoder_outputs, np.float32)
    dec = np.asarray(decoder_h, np.float32)[:, 0, :]
    prev_alpha = np.asarray(prev_alpha, np.float32)
    noise = np.asarray(noise, np.float32)

    c_m = float(np.asarray(mvb)[0] + np.asarray(mr)[0])
    c_c = float(np.asarray(cvb)[0] + np.asarray(cr)[0])
    vg_m = float(np.asarray(mvg)[0])
    vg_c = float(np.asarray(cvg)[0])

    nc = _build(c_m, c_c, vg_m, vg_c)

    # weights: wT[p, k*256 + br*128 + a] = W_br[a, k*128+p]
    def packT(wm, wc, dtype):
        out = np.empty((128, ND, 256), np.float32)
        wmT = np.asarray(wm, np.float32).T  # [D, A]
        wcT = np.asarray(wc, np.float32).T
        for k in range(ND):
            out[:, k, 0:128] = wmT[k * 128:(k + 1) * 128]
            out[:, k, 128:256] = wcT[k * 128:(k + 1) * 128]
        return np.ascontiguousarray(out.reshape(128, ND * 256).astype(dtype))

    host_consts = {
        "wt": packT(mW, cW, BF16NP),
        "vt": packT(mV, cV, BF16NP),
        "cst": _scan_consts().astype(BF16NP),
        "bmc": np.ascontiguousarray(
            np.stack([np.asarray(mb, np.float32),
                      np.asarray(cb, np.float32)], axis=1)),
        "vv": np.ascontiguousarray(
            np.concatenate([np.asarray(mvv, np.float32)[0],
                            np.asarray(cvv, np.float32)[0]])[None, :]),
    }

    in_maps = []
    for i in range(NCORES):
        sl = slice(i * BL, (i + 1) * BL)
        in_maps.append(_prep_core_inputs(
            encoder_outputs[sl], dec[sl], prev_alpha[sl], noise[sl],
            host_consts))

    global LAST_RESULT
    res = bass_utils.run_bass_kernel_spmd(
        nc, in_maps, core_ids=list(range(NCORES)), trace=TRACE,
        **RUN_KWARGS)
    LAST_RESULT = res

    ctx = np.empty((B, D), np.float32)
    alpha = np.empty((B, S), np.float32)
    beta = np.empty((B, S), np.float32)
    for i in range(NCORES):
        r = res.results[i]
        ctx[i * BL:(i + 1) * BL] = r["cto"]
        # [128, BL*NSC] -> [BL, S]: value at (p, b*NSC+c) is s = c*128+p
        for name, dst in (("alo", alpha), ("beo", beta)):
            x = r[name].reshape(128, BL, NSC).transpose(1, 2, 0).reshape(BL, S)
            dst[i * BL:(i + 1) * BL] = x
    return ctx, alpha, beta


TRACE = False
RUN_KWARGS: dict = {}
LAST_RESULT = None
